# revision 13
# baseline (speedup 1.0000x reference)
"""DLightGCN (LightGCN propagation + disentangled-factor scoring) on 8 trn2
NeuronCores via Bass/Tile.

Sharding: edge list and segment-sum sharded by destination-node partition
(core i owns padded node rows [i*R, (i+1)*R)); per-layer node features are
exchanged with an on-device AllGather; factor weights replicated; the
(user,item) batch data-parallel across cores.

Per destination tile of 128 rows, edges are packed into chunks of 128 (one
edge per SBUF partition). Chunk counts per tile are data-derived (max over
cores) and baked into the program, removing the fixed-CPT padding. Features
are bf16 end-to-end in the propagation (PSUM accumulation stays f32): halves
gather payloads and runs the chunk matmuls at full PE rate. The per-chunk
selection matrix is built with a single-port DVE tensor_scalar
(iota==lr)*ev in 2x mode, which avoids the 2-port perf-mode lock on the
shared DVE<->GpSimd SBUF port that the Q7 SWDGE needs while generating
gather descriptors.

kernel(**inputs) takes the FULL problem inputs and returns the FULL [B]
scores; all sharding happens inside.
"""
import sys

import numpy as np

for _p in ("/opt/trn_rl_repo", "/root/.axon_site/_ro/trn_rl_repo"):
    if _p not in sys.path:
        sys.path.append(_p)

import ml_dtypes  # noqa: E402

import concourse.bass as bass  # noqa: E402
import concourse.mybir as mybir  # noqa: E402
from concourse.bass import IndirectOffsetOnAxis  # noqa: E402

F32 = mybir.dt.float32
BF16 = mybir.dt.bfloat16
I32 = mybir.dt.int32
AF = mybir.ActivationFunctionType
OP = mybir.AluOpType

N_CORES = 8
D = 128
K = 4
L = 3
T = 147           # dest tiles per core
BT = 16           # batch tiles (of 128 pairs) per core
R = T * 128
N_PAD = N_CORES * R
N_USERS = 100000


def body(tc, outs, ins, cpw, cpw3):
    nc = tc.nc
    off = np.concatenate([[0], np.cumsum(cpw)]).astype(int)
    TC = int(off[-1])
    off3 = np.concatenate([[0], np.cumsum(cpw3)]).astype(int)
    TC3 = int(off3[-1])
    scores = outs["scores"]
    rg = [list(range(N_CORES))]
    xb0 = ins["xb0"]

    with tc.tile_pool(name="dram", bufs=1, space="DRAM") as dpool:
        y1 = dpool.tile([R, D], BF16)
        y2 = dpool.tile([R, D], BF16)
        light_sl = dpool.tile([R, D], BF16)
        x1f = dpool.tile([N_PAD, D], BF16, addr_space="Shared")
        x2f = dpool.tile([N_PAD, D], BF16, addr_space="Shared")
        lightf = dpool.tile([N_PAD, D], BF16, addr_space="Shared")

        with (
            tc.tile_pool(name="cpool", bufs=1) as cpool,
            tc.tile_pool(name="apool", bufs=1) as apool,
            tc.tile_pool(name="gpool", bufs=24) as gpool,
            tc.tile_pool(name="vpool", bufs=24) as vpool,
            tc.tile_pool(name="pspool", bufs=4, space="PSUM") as pspool,
            tc.tile_pool(name="epool", bufs=8) as epool,
        ):
            idx_sb = cpool.tile([128, TC], I32)
            nc.sync.dma_start(idx_sb[:], ins["src_idx"][:])
            lr_sb = cpool.tile([128, TC], F32)
            nc.sync.dma_start(lr_sb[:], ins["lr"][:])
            ev_sb = cpool.tile([128, TC], F32)
            nc.sync.dma_start(ev_sb[:], ins["ev"][:])
            idx3_sb = cpool.tile([128, TC3], I32)
            nc.sync.dma_start(idx3_sb[:], ins["src_idx3"][:])
            lr3_sb = cpool.tile([128, TC3], F32)
            nc.sync.dma_start(lr3_sb[:], ins["lr3"][:])
            ev3_sb = cpool.tile([128, TC3], F32)
            nc.sync.dma_start(ev3_sb[:], ins["ev3"][:])
            iota_sb = cpool.tile([128, 128], BF16)
            nc.sync.dma_start(iota_sb[:], ins["iota"][:])
            acc = apool.tile([128, T * 128], F32)

            xs = [xb0, x1f, x2f]
            for layer in range(L):
                xsrc = xs[layer]
                last = layer == L - 1
                l_cpw = cpw3 if last else cpw
                l_off = off3 if last else off
                l_idx, l_lr, l_ev = ((idx3_sb, lr3_sb, ev3_sb) if last
                                     else (idx_sb, lr_sb, ev_sb))
                for t in range(T):
                    ps = pspool.tile([128, D], F32, name=f"ps_{layer}_{t}",
                                     tag="ps")
                    for c in range(l_cpw[t]):
                        cc = int(l_off[t]) + c
                        g = gpool.tile([128, D], BF16,
                                       name=f"g_{layer}_{t}_{c}", tag="g")
                        nc.gpsimd.indirect_dma_start(
                            out=g[:], out_offset=None, in_=xsrc[:],
                            in_offset=IndirectOffsetOnAxis(
                                ap=l_idx[:, cc:cc + 1], axis=0))
                        vh = vpool.tile([128, 128], BF16,
                                        name=f"vh_{layer}_{t}_{c}", tag="vh")
                        nc.vector.tensor_scalar(
                            out=vh[:], in0=iota_sb[:],
                            scalar1=l_lr[:, cc:cc + 1],
                            scalar2=l_ev[:, cc:cc + 1],
                            op0=OP.is_equal, op1=OP.mult)
                        nc.tensor.matmul(
                            ps[:], lhsT=vh[:], rhs=g[:],
                            start=(c == 0), stop=(c == l_cpw[t] - 1))
                    a_sl = acc[:, t * 128:(t + 1) * 128]
                    if layer == 0:
                        ld = epool.tile([128, D], BF16,
                                        name=f"ld_{t}", tag="ld")
                        nc.sync.dma_start(
                            ld[:], ins["x0own"][t * 128:(t + 1) * 128, :])
                        yt = epool.tile([128, D], BF16,
                                        name=f"yt0_{t}", tag="yt")
                        nc.scalar.activation(yt[:], ps[:], AF.Copy)
                        nc.sync.dma_start(y1[t * 128:(t + 1) * 128, :], yt[:])
                        nc.vector.tensor_tensor(out=a_sl, in0=ld[:], in1=ps[:],
                                                op=OP.add)
                    elif layer == 1:
                        yt = epool.tile([128, D], BF16,
                                        name=f"yt1_{t}", tag="yt")
                        nc.scalar.activation(yt[:], ps[:], AF.Copy)
                        nc.sync.dma_start(y2[t * 128:(t + 1) * 128, :], yt[:])
                        nc.vector.tensor_tensor(out=a_sl, in0=a_sl, in1=ps[:],
                                                op=OP.add)
                    else:
                        lt = epool.tile([128, D], BF16,
                                        name=f"lt_{t}", tag="yt")
                        nc.vector.tensor_tensor(out=lt[:], in0=a_sl, in1=ps[:],
                                                op=OP.add)
                        nc.sync.dma_start(
                            light_sl[t * 128:(t + 1) * 128, :], lt[:])
                if layer == 0:
                    nc.gpsimd.collective_compute(
                        "AllGather", OP.bypass, ins=[y1.opt()],
                        outs=[x1f.opt()], replica_groups=rg)
                elif layer == 1:
                    nc.gpsimd.collective_compute(
                        "AllGather", OP.bypass, ins=[y2.opt()],
                        outs=[x2f.opt()], replica_groups=rg)
            nc.gpsimd.collective_compute(
                "AllGather", OP.bypass, ins=[light_sl.opt()],
                outs=[lightf.opt()], replica_groups=rg)

        # ---- batch stage ----
        with (
            tc.tile_pool(name="bcpool", bufs=1) as bcpool,
            tc.tile_pool(name="bpool", bufs=3) as bpool,
            tc.tile_pool(name="bps", bufs=2, space="PSUM") as bps,
        ):
            wft_sb = bcpool.tile([128, K * D], BF16)
            nc.sync.dma_start(wft_sb[:], ins["wft"][:])
            bias_sb = bcpool.tile([128, K * D], F32)
            nc.sync.dma_start(bias_sb[:], ins["bias"][:])
            ws_sb = bcpool.tile([128, K * K], F32)
            nc.sync.dma_start(ws_sb[:], ins["ws"][:])
            ident_sb = bcpool.tile([128, 128], BF16)
            nc.sync.dma_start(ident_sb[:], ins["identity"][:])
            u_idx = bcpool.tile([128, BT], I32)
            nc.sync.dma_start(u_idx[:], ins["users_idx"][:])
            i_idx = bcpool.tile([128, BT], I32)
            nc.sync.dma_start(i_idx[:], ins["items_idx"][:])
            sc = bcpool.tile([128, BT], F32)
            dmp = bcpool.tile([128, 128], BF16)
            dmp16 = bcpool.tile([128, K * K], F32)

            for tb in range(BT):
                fs = []
                for side, sidx in (("u", u_idx), ("i", i_idx)):
                    e = bpool.tile([128, D], BF16, name=f"e{side}_{tb}",
                                   tag=f"e{side}")
                    nc.gpsimd.indirect_dma_start(
                        out=e[:], out_offset=None, in_=lightf[:],
                        in_offset=IndirectOffsetOnAxis(
                            ap=sidx[:, tb:tb + 1], axis=0))
                    pt = bps.tile([128, 128], BF16, name=f"pt{side}_{tb}",
                                  tag="pt")
                    nc.tensor.transpose(pt[:], e[:], ident_sb[:])
                    eT = bpool.tile([128, D], BF16, name=f"eT{side}_{tb}",
                                    tag=f"eT{side}")
                    nc.scalar.activation(eT[:], pt[:], AF.Copy)
                    fp = bps.tile([128, K * D], F32, name=f"fp{side}_{tb}",
                                  tag="fp")
                    nc.tensor.matmul(fp[:], lhsT=eT[:], rhs=wft_sb[:],
                                     start=True, stop=True)
                    f = bpool.tile([128, K * D], F32, name=f"f{side}_{tb}",
                                   tag=f"f{side}")
                    nc.vector.tensor_tensor(out=f[:], in0=fp[:],
                                            in1=bias_sb[:], op=OP.add)
                    fb = bpool.tile([128, K * D], BF16, name=f"fb{side}_{tb}",
                                    tag=f"fb{side}")
                    nc.vector.tensor_scalar(out=fb[:], in0=f[:], scalar1=0.0,
                                            scalar2=None, op0=OP.max)
                    n2 = bpool.tile([128, K], F32, name=f"n2{side}_{tb}",
                                    tag=f"n2{side}")
                    sq = bpool.tile([128, K * D], BF16, name=f"sq{side}_{tb}",
                                    tag="sq")
                    nc.vector.tensor_tensor(out=sq[:], in0=fb[:], in1=fb[:],
                                            op=OP.mult)
                    for k in range(K):
                        nc.vector.reduce_sum(out=n2[:, k:k + 1],
                                             in_=sq[:, k * D:(k + 1) * D],
                                             axis=mybir.AxisListType.X)
                    nc.vector.tensor_scalar(out=n2[:], in0=n2[:],
                                            scalar1=1e-24, scalar2=None,
                                            op0=OP.max)
                    nrm = bpool.tile([128, K], F32, name=f"nr{side}_{tb}",
                                     tag=f"nr{side}")
                    nc.scalar.activation(nrm[:], n2[:], AF.Sqrt)
                    inv = bpool.tile([128, K], F32, name=f"iv{side}_{tb}",
                                     tag=f"iv{side}")
                    nc.vector.reciprocal(inv[:], nrm[:])
                    for k in range(K):
                        nc.vector.tensor_scalar(
                            out=fb[:, k * D:(k + 1) * D],
                            in0=fb[:, k * D:(k + 1) * D],
                            scalar1=inv[:, k:k + 1], scalar2=None,
                            op0=OP.mult)
                    fs.append(fb)
                uf, itf = fs
                h = bpool.tile([128, K * K], F32, name=f"h_{tb}", tag="h")
                for i in range(K):
                    for j in range(K):
                        nc.vector.tensor_tensor(
                            out=dmp[:], in0=uf[:, i * D:(i + 1) * D],
                            in1=itf[:, j * D:(j + 1) * D], op=OP.mult)
                        nc.vector.reduce_sum(
                            out=h[:, i * K + j:i * K + j + 1],
                            in_=dmp[:], axis=mybir.AxisListType.X)
                nc.vector.tensor_tensor(out=dmp16[:], in0=h[:], in1=ws_sb[:],
                                        op=OP.mult)
                nc.vector.reduce_sum(out=sc[:, tb:tb + 1], in_=dmp16[:],
                                     axis=mybir.AxisListType.X)
            nc.sync.dma_start(scores[:], sc[:])


def build_full(cpw, cpw3):
    import concourse.bacc as bacc
    import concourse.tile as tile_mod
    nc = bacc.Bacc("TRN2", target_bir_lowering=False, debug=False,
                   num_devices=N_CORES)
    TC = int(np.sum(cpw))
    TC3 = int(np.sum(cpw3))
    shapes = dict(
        xb0=([N_PAD, D], BF16), x0own=([R, D], BF16),
        src_idx=([128, TC], I32),
        lr=([128, TC], F32), ev=([128, TC], F32),
        src_idx3=([128, TC3], I32),
        lr3=([128, TC3], F32), ev3=([128, TC3], F32),
        iota=([128, 128], BF16), identity=([128, 128], BF16),
        wft=([128, K * D], BF16), bias=([128, K * D], F32),
        ws=([128, K * K], F32),
        users_idx=([128, BT], I32), items_idx=([128, BT], I32),
    )
    ins = {k: nc.dram_tensor(k, s, d, kind="ExternalInput").ap()
           for k, (s, d) in shapes.items()}
    outs = {"scores": nc.dram_tensor("scores", [128, BT], F32,
                                     kind="ExternalOutput").ap()}
    with tile_mod.TileContext(nc) as tc:
        body(tc, outs, ins, cpw, cpw3)
    nc.compile()
    return nc


def _needed_mask(users, items, n_users):
    need = np.zeros(N_PAD, bool)
    need[np.asarray(users).astype(np.int64)] = True
    need[np.asarray(items).astype(np.int64) + n_users] = True
    return need


def host_structure(edge_index, users, items, n_users):
    rows = np.asarray(edge_index[0]).astype(np.int64)
    gt = rows // 128
    counts = np.bincount(gt, minlength=N_CORES * T).reshape(N_CORES, T)
    cpw = np.maximum(1, -(-counts.max(axis=0) // 128)).astype(int)
    need = _needed_mask(users, items, n_users)
    rows3 = rows[need[rows]]
    gt3 = rows3 // 128
    counts3 = np.bincount(gt3, minlength=N_CORES * T).reshape(N_CORES, T)
    cpw3 = np.maximum(1, -(-counts3.max(axis=0) // 128)).astype(int)
    return cpw, cpw3


def _edge_arrays(rs, cs, vs, cpw, off, TC):
    """rs/cs/vs sorted by rs. Returns per-core [128, TC] idx/lr/ev."""
    gt = rs // 128
    counts = np.bincount(gt, minlength=N_CORES * T)
    starts = np.zeros(N_CORES * T, np.int64)
    starts[1:] = np.cumsum(counts)[:-1]
    pos = np.arange(len(rs)) - starts[gt]
    chunk = pos // 128
    lane = pos % 128
    core_of = gt // T
    t_in_core = gt % T
    colidx = off[t_in_core] + chunk
    assert (chunk < cpw[t_in_core]).all()
    src = np.zeros((N_CORES, 128, TC), np.int32)
    lr = np.zeros((N_CORES, 128, TC), np.float32)
    ev = np.zeros((N_CORES, 128, TC), np.float32)
    src[core_of, lane, colidx] = cs
    lr[core_of, lane, colidx] = (rs % 128).astype(np.float32)
    ev[core_of, lane, colidx] = vs
    return src, lr, ev


def host_prepare(inputs, cpw, cpw3):
    users = np.asarray(inputs["users"])
    items = np.asarray(inputs["items"])
    edge_index = np.asarray(inputs["edge_index"])
    edge_vals = np.asarray(inputs["edge_vals"], dtype=np.float32)
    user_emb = np.asarray(inputs["user_emb"], dtype=np.float32)
    item_emb = np.asarray(inputs["item_emb"], dtype=np.float32)
    W_f = np.asarray(inputs["W_f"], dtype=np.float32)
    b_f = np.asarray(inputs["b_f"], dtype=np.float32)
    W_s = np.asarray(inputs["W_s"], dtype=np.float32)

    n_users = user_emb.shape[0]
    N = n_users + item_emb.shape[0]
    assert N <= N_PAD, (N, N_PAD)
    B = users.shape[0]
    assert B == N_CORES * BT * 128, (B, N_CORES, BT)

    all_emb = np.zeros((N_PAD, D), np.float32)
    all_emb[:n_users] = user_emb
    all_emb[n_users:N] = item_emb
    all_bf = all_emb.astype(ml_dtypes.bfloat16)

    off = np.concatenate([[0], np.cumsum(cpw)]).astype(np.int64)
    TC = int(off[-1])
    off3 = np.concatenate([[0], np.cumsum(cpw3)]).astype(np.int64)
    TC3 = int(off3[-1])

    rows = edge_index[0].astype(np.int64)
    cols = edge_index[1].astype(np.int64)
    order = np.argsort(rows, kind="stable")
    rs, cs, vs = rows[order], cols[order], edge_vals[order]
    src, lr, ev = _edge_arrays(rs, cs, vs, cpw, off, TC)

    need = _needed_mask(users, items, n_users)
    m3 = need[rs]
    src3, lr3, ev3 = _edge_arrays(rs[m3], cs[m3], vs[m3], cpw3, off3, TC3)

    iota = np.tile(np.arange(128, dtype=np.float32),
                   (128, 1)).astype(ml_dtypes.bfloat16)
    ident = np.eye(128, dtype=np.float32).astype(ml_dtypes.bfloat16)
    wft = np.transpose(W_f, (2, 0, 1)).reshape(D, K * D).astype(
        ml_dtypes.bfloat16)
    bias = np.tile(b_f.reshape(1, K * D), (128, 1)).astype(np.float32)
    ws = np.tile(W_s.reshape(1, K * K), (128, 1)).astype(np.float32)

    in_maps = []
    for c in range(N_CORES):
        u_sh = users[c * BT * 128:(c + 1) * BT * 128].astype(np.int32)
        i_sh = items[c * BT * 128:(c + 1) * BT * 128].astype(np.int32) \
            + n_users
        in_maps.append(dict(
            xb0=all_bf,
            x0own=all_bf[c * R:(c + 1) * R].copy(),
            src_idx=src[c], lr=lr[c], ev=ev[c],
            src_idx3=src3[c], lr3=lr3[c], ev3=ev3[c],
            iota=iota, identity=ident, wft=wft, bias=bias, ws=ws,
            users_idx=np.ascontiguousarray(u_sh.reshape(BT, 128).T),
            items_idx=np.ascontiguousarray(i_sh.reshape(BT, 128).T),
        ))
    return in_maps


def host_post(results):
    outs = []
    for c in range(N_CORES):
        arr = results[c]["scores"]  # [128, BT]
        outs.append(arr.T.reshape(-1))
    return np.concatenate(outs)


_CACHE = {}


def kernel(**inputs) -> np.ndarray:
    from concourse import bass_utils

    n_users = np.asarray(inputs["user_emb"]).shape[0]
    cpw, cpw3 = host_structure(np.asarray(inputs["edge_index"]),
                               inputs["users"], inputs["items"], n_users)
    key = ("nc", tuple(cpw.tolist()), tuple(cpw3.tolist()))
    in_maps = host_prepare(inputs, cpw, cpw3)
    _CACHE["cpw"] = (cpw, cpw3)
    nc = _CACHE.get(key)
    if nc is None:
        nc = build_full(cpw, cpw3)
        _CACHE[key] = nc
    _CACHE["nc"] = nc
    res = bass_utils.run_bass_kernel_spmd(
        nc, in_maps, core_ids=list(range(N_CORES)))
    return host_post(res.results).astype(np.float32)
